# revision 27
# baseline (speedup 1.0000x reference)
"""DetectionLoss Trainium2 kernel (8-core data parallel), v3.

Per core: 64 samples; groups of 2 samples -> partitions = (sample-in-pair,
64 targets); free dim = 1176 preds.

Score phase per group: PE broadcasts 5 raw per-sample rows (bx2, bx1, by2,
by1, ab) via a single constant K=2 selector lhsT (one LDW amortized); ACT
evacuates with fused per-partition target biases using the identity
  dx = min(bx2,tx2) - max(bx1,tx1) = wt - relu(tx2-bx2) - relu(bx1-tx1)
so r1 = Relu(-bx2 + tx2), r2 = Relu(bx1 - tx1); then
  fx = (r1 - wt) + r2 = -dx,  min(fx,0)*min(fy,0) = relu(dx)*relu(dy) = inter
score S = inter / (ab + at) is a strictly increasing function of IoU
(S = IoU/(1+IoU)), so hardware max + first-occurrence max_index reproduces
jnp.argmax exactly, including the all-zero-IoU tie case (index 0).

Gather phase (deferred one group for pipelining): matched-index row is
broadcast (transpose + ones-matmul), one-hot M_j per 128-chunk, 10
accumulating matmuls gather the raw 9 pred values per (sample, target).
Losses (smooth-l1 on decoded gathered boxes, CE on gathered logits,
BCE-with-logits via Softplus sum minus sum of conf over distinct matched
preds) reduce per-partition, then across partitions via a ones matmul.
"""

import os
import numpy as np
from contextlib import ExitStack

import concourse.bass as bass
import concourse.mybir as mybir
from concourse import bacc, tile
from concourse.bass_utils import run_bass_kernel_spmd
from concourse.masks import make_identity

F32 = mybir.dt.float32
F32R = mybir.dt.float32r
I32 = mybir.dt.int32
U32 = mybir.dt.uint32
OP = mybir.AluOpType
AF = mybir.ActivationFunctionType
AX = mybir.AxisListType

B, N, T, C = 512, 1176, 64, 4
NCORES = 8
BC = B // NCORES          # samples per core = 64
NG = BC // 2              # groups of 2 samples = 32
NJ = 10                   # 128-chunks of N (last has 24 valid rows)
NTAIL = N - 9 * 128       # 24
CHUNKS = [(0, 392), (392, 392), (784, 392)]
IMG_W, IMG_H = 1472.0, 832.0
LN16 = float(np.log(np.float64(16.0)))
SQRT_HALF = float(np.sqrt(np.float64(0.5)))

TTR_DIV = int(os.environ.get("K_TTRDIV", "1"))
USE_SOFTPLUS = int(os.environ.get("K_SOFTPLUS", "0"))
GP_FXY = int(os.environ.get("K_GPFXY", "1"))


def r32(ap):
    return ap.bitcast(F32R)


def build_kernel():
    nc = bacc.Bacc(
        "TRN2",
        target_bir_lowering=False,
        debug=False,
        enable_asserts=False,
        num_devices=NCORES,
    )
    pred_d = nc.dram_tensor("predictions", [BC, N, 9], F32, kind="ExternalInput").ap()
    tb_d = nc.dram_tensor("target_boxes", [BC, T, 4], F32, kind="ExternalInput").ap()
    tc_d = nc.dram_tensor("target_classes", [BC, T], I32, kind="ExternalInput").ap()
    selg_d = nc.dram_tensor("selg", [64, NG * 128], F32R, kind="ExternalInput").ap()
    out_d = nc.dram_tensor("out", [3], F32, kind="ExternalOutput").ap()

    with tile.TileContext(nc) as tcx:
        with ExitStack() as ctx:
            emit(ctx, tcx, pred_d, tb_d, tc_d, selg_d, out_d)
    nc.compile()
    return nc


def emit(ctx, tcx, pred_d, tb_d, tc_d, selg_d, out_d):
    nc = tcx.nc
    tp = lambda name, bufs, **kw: ctx.enter_context(
        tcx.tile_pool(name=name, bufs=bufs, **kw)
    )
    vec = nc.vector
    act = nc.scalar
    gp = nc.gpsimd

    const_p = tp("const", 1)
    big_p = tp("big", 1)
    work_p = tp("work", 2)
    small_p = tp("small", 3)
    ps_p = tp("ps", 2, space="PSUM")
    mall_p = tp("mall", 1)

    # ---------------- constants ----------------
    ident = const_p.tile([128, 128], F32, tag="ident")
    make_identity(nc, ident[:, :])
    ones1 = const_p.tile([1, 128], F32, tag="ones1")
    vec.memset(ones1[:, :], 1.0)
    onescol = const_p.tile([128, 1], F32, tag="onescol")
    vec.memset(onescol[:, :], 1.0)

    cbias = const_p.tile([128, 4], F32, tag="cbias")
    vec.memset(cbias[:, 0:1], LN16)
    vec.memset(cbias[:, 1:2], -IMG_W / 2)
    vec.memset(cbias[:, 2:3], -IMG_H / 2)
    vec.memset(cbias[:, 3:4], -1.0)
    b_ln16, b_wneg, b_hneg, b_neg1 = (cbias[:, i : i + 1] for i in range(4))

    # SELG[k, g, p] = 1 iff k == 2g + (p >= 64): per-group K=64 sample selector
    SELG = const_p.tile([64, NG, 128], F32R, tag="SELG")
    nc.sync.dma_start(
        SELG[:, :, :], selg_d.rearrange("k (g p) -> k g p", p=128)
    )

    njcol_i = const_p.tile([128, NJ], I32, tag="njcol_i")
    gp.iota(njcol_i[:, :], pattern=[[128, NJ]], base=0, channel_multiplier=1)
    NJCOL = const_p.tile([128, NJ], F32, tag="NJCOL")
    vec.tensor_copy(NJCOL[:, :], njcol_i[:, :])

    rowv_i = const_p.tile([128, 1], I32, tag="rowv_i")
    gp.iota(rowv_i[:, :], pattern=[[0, 1]], base=0, channel_multiplier=1)
    ROWV = const_p.tile([128, 1], F32, tag="ROWV")
    vec.tensor_copy(ROWV[:, :], rowv_i[:, :])
    coli_i = const_p.tile([128, 128], I32, tag="coli_i")
    gp.iota(coli_i[:, :], pattern=[[1, 128]], base=0, channel_multiplier=0)
    COLI = const_p.tile([128, 128], F32, tag="COLI")
    vec.tensor_copy(COLI[:, :], coli_i[:, :])
    # SMASK[p, c] = (c < p) and (c // 64 == p // 64)
    SMA = const_p.tile([128, 128], F32, tag="SMA")
    vec.tensor_scalar(SMA[:, :], COLI[:, :], ROWV[:, :], None, OP.is_lt)
    SMC = const_p.tile([128, 128], F32, tag="SMC")
    vec.tensor_scalar(SMC[:, :], COLI[:, :], 64.0, None, OP.is_lt)
    SMP = const_p.tile([128, 1], F32, tag="SMP")
    vec.tensor_scalar(SMP[:, :], ROWV[:, :], 64.0, None, OP.is_lt)
    vec.tensor_scalar(SMC[:, :], SMC[:, :], SMP[:, :], None, OP.is_equal)
    SMASK = const_p.tile([128, 128], F32, tag="SMASK")
    vec.tensor_tensor(SMASK[:, :], SMA[:, :], SMC[:, :], OP.mult)

    # ---------------- loads ----------------
    # XN[p, s, j, k] = pred[s, 128j+p, k]; pad rows zeroed, conf -> -100
    XN = big_p.tile([128, BC, NJ, 9], F32, tag="XN")
    vec.memset(XN[:, :, 9, :], 0.0)
    vec.memset(XN[:, :, 9, 4], -100.0)
    for i in range(2):
        s0 = i * 32
        for j in range(NJ):
            pw = 128 if j < 9 else NTAIL
            eng = [nc.sync, nc.gpsimd, nc.scalar][(i * NJ + j) % 3]
            eng.dma_start(
                XN[0:pw, s0 : s0 + 32, j, :],
                pred_d[s0 : s0 + 32, j * 128 : j * 128 + pw, :].rearrange(
                    "s p k -> p s k"
                ),
            )

    TGTC = big_p.tile([128, NG, 4], F32, tag="TGTC")
    nc.sync.dma_start(TGTC[:, :, :], tb_d.rearrange("(g s) t c -> (s t) g c", s=2))
    TCI = big_p.tile([128, NG], I32, tag="TCI")
    nc.sync.dma_start(TCI[:, :], tc_d.rearrange("(g s) t -> (s t) g", s=2))
    TCF = big_p.tile([128, NG], F32, tag="TCF")
    vec.tensor_copy(TCF[:, :], TCI[:, :])

    # target-derived per-partition scalars
    WT = big_p.tile([128, NG], F32, tag="WT")
    HT = big_p.tile([128, NG], F32, tag="HT")
    AT = big_p.tile([128, NG], F32, tag="AT")
    NTX1 = big_p.tile([128, NG], F32, tag="NTX1")
    NTY1 = big_p.tile([128, NG], F32, tag="NTY1")
    vec.tensor_tensor(WT[:, :], TGTC[:, :, 2], TGTC[:, :, 0], OP.subtract)
    vec.tensor_tensor(HT[:, :], TGTC[:, :, 3], TGTC[:, :, 1], OP.subtract)
    vec.tensor_tensor(AT[:, :], WT[:, :], HT[:, :], OP.mult)
    vec.tensor_scalar(NTX1[:, :], TGTC[:, :, 0], -1.0, None, OP.mult)
    vec.tensor_scalar(NTY1[:, :], TGTC[:, :, 1], -1.0, None, OP.mult)

    # conf-loss softplus sum over all preds (pad rows conf=-100 -> ~0)
    SPACC = big_p.tile([128, 1], F32, tag="SPACC")
    SPX = work_p.tile([128, BC * NJ], F32, tag="SALL")
    x4flat = XN[:, :, :, 4].rearrange("p s j -> p (s j)")
    if USE_SOFTPLUS:
        act.activation(SPX[:, :], x4flat, AF.Softplus)
        vec.tensor_reduce(SPACC[:, :], SPX[:, :], AX.X, OP.add)
    else:
        SPR = work_p.tile([128, BC * NJ], F32, tag="IP")
        act.activation(SPX[:, :], x4flat, AF.Abs)
        act.activation(SPX[:, :], SPX[:, :], AF.Exp, scale=-1.0)
        act.activation(SPX[:, :], SPX[:, :], AF.Ln, bias=onescol[:, :])
        act.activation(SPR[:, :], x4flat, AF.Relu)
        vec.scalar_tensor_tensor(
            SPX[:, :], SPX[:, :], 0.0, SPR[:, :], OP.add, OP.add,
            accum_out=SPACC[:, :],
        )

    # ---------------- sample-major decode (two half-width passes) ----------------
    DALL = big_p.tile([64, 5, N], F32R, tag="DALL")  # bx2, bx1, by2, by1, ab
    HW_ = N // 6
    with tcx.tile_pool(name="x2pool", bufs=1) as x2_p:
        for h in range(6):
            n0 = h * HW_
            ns = slice(n0, n0 + HW_)
            X2 = x2_p.tile([64, HW_, 9], F32, tag="X2")
            nc.sync.dma_start(X2[:, :, :], pred_d[:, ns, :])
            E2W = work_p.tile([64, HW_], F32, tag="R1X")
            E2H = work_p.tile([64, HW_], F32, tag="R2X")
            C2X = work_p.tile([64, HW_], F32, tag="R1Y")
            C2Y = work_p.tile([64, HW_], F32, tag="R2Y")
            act.activation(E2W[:, :], X2[:, :, 2], AF.Exp, bias=b_ln16[0:64, :])
            act.activation(E2H[:, :], X2[:, :, 3], AF.Exp, bias=b_ln16[0:64, :])
            act.activation(C2X[:, :], X2[:, :, 0], AF.Identity,
                           bias=b_wneg[0:64, :], scale=IMG_W)
            act.activation(C2Y[:, :], X2[:, :, 1], AF.Identity,
                           bias=b_hneg[0:64, :], scale=IMG_H)
            vec.tensor_tensor(DALL[:, 0, ns], C2X[:, :], E2W[:, :], OP.add)
            vec.tensor_tensor(DALL[:, 1, ns], C2X[:, :], E2W[:, :], OP.subtract)
            vec.tensor_tensor(DALL[:, 2, ns], C2Y[:, :], E2H[:, :], OP.add)
            vec.tensor_tensor(DALL[:, 3, ns], C2Y[:, :], E2H[:, :], OP.subtract)
            ABW = work_p.tile([64, HW_], F32, tag="SABR")
            vec.tensor_tensor(ABW[:, :], E2W[:, :], E2H[:, :], OP.mult)
            vec.tensor_scalar(DALL[:, 4, ns], ABW[:, :], 4.0, None, OP.mult)

    # persistent per-group results
    VM = big_p.tile([128, NG], F32, tag="VM")
    MF = big_p.tile([128, NG], F32, tag="MF")
    CB = big_p.tile([128, NG], F32, tag="CB")
    GALL = big_p.tile([128, NG, 9, 2], F32, tag="GALL")
    JK = big_p.tile([128, 128], F32, tag="JK")

    # ---------------- per-group score (+ deferred gather) ----------------
    def emit_score(g):
        R1X = work_p.tile([128, N], F32, tag="R1X")
        R2X = work_p.tile([128, N], F32, tag="R2X")
        R1Y = work_p.tile([128, N], F32, tag="R1Y")
        R2Y = work_p.tile([128, N], F32, tag="R2Y")
        SABR = work_p.tile([128, N], F32, tag="SABR")
        for c0, cw in CHUNKS:
            qt = ps_p.tile([128, 5, 392], F32, tag="qt")
            for q in range(5):
                nc.tensor.matmul(
                    qt[:, q, 0:cw],
                    SELG[:, g, :],
                    DALL[:, q, c0 : c0 + cw],
                    start=True,
                    stop=True,
                )
            cs = slice(c0, c0 + cw)
            act.activation(R1X[:, cs], qt[:, 0, 0:cw], AF.Relu,
                           bias=TGTC[:, g : g + 1, 2], scale=-1.0)
            act.activation(R2X[:, cs], qt[:, 1, 0:cw], AF.Relu,
                           bias=NTX1[:, g : g + 1])
            act.activation(R1Y[:, cs], qt[:, 2, 0:cw], AF.Relu,
                           bias=TGTC[:, g : g + 1, 3], scale=-1.0)
            act.activation(R2Y[:, cs], qt[:, 3, 0:cw], AF.Relu,
                           bias=NTY1[:, g : g + 1])
            act.activation(SABR[:, cs], qt[:, 4, 0:cw], AF.Identity,
                           bias=AT[:, g : g + 1])
        # ex = r1 + r2 (gp);  nf = relu(wt - ex) = relu(dx) (ACT, bias=wt)
        EX = work_p.tile([128, N], F32, tag="FX")
        EY = work_p.tile([128, N], F32, tag="FY")
        feng = gp if GP_FXY else vec
        feng.tensor_tensor(EX[:, :], R1X[:, :], R2X[:, :], OP.add)
        feng.tensor_tensor(EY[:, :], R1Y[:, :], R2Y[:, :], OP.add)
        NFX = work_p.tile([128, N], F32, tag="NFX")
        NFY = work_p.tile([128, N], F32, tag="MFY")
        act.activation(NFX[:, :], EX[:, :], AF.Relu, bias=WT[:, g : g + 1],
                       scale=-1.0)
        act.activation(NFY[:, :], EY[:, :], AF.Relu, bias=HT[:, g : g + 1],
                       scale=-1.0)
        IP = work_p.tile([128, N], F32, tag="IP")
        vec.tensor_tensor(IP[:, :], NFX[:, :], NFY[:, :], OP.mult)
        RS = work_p.tile([128, N], F32, tag="RS")
        vec.reciprocal_approx_fast(RS[:, :], SABR[:, :])
        SALL = work_p.tile([128, N], F32, tag="SALL")
        vec.tensor_tensor(SALL[:, :], IP[:, :], RS[:, :], OP.mult)
        vec.tensor_reduce(VM[:, g : g + 1], SALL[:, :], AX.X, OP.max)
        v8 = small_p.tile([128, 8], F32, tag="v8")
        vec.tensor_scalar(v8[:, :], SALL[:, 0:8], 0.0, VM[:, g : g + 1],
                          OP.mult, OP.add)
        idx8 = small_p.tile([128, 8], U32, tag="idx8")
        vec.max_index(idx8[:, :], v8[:, :], SALL[:, :])
        vec.tensor_copy(MF[:, g : g + 1], idx8[:, 0:1])

    def emit_gather(g):
        # broadcast matched index over partitions
        mrow_ps = ps_p.tile([1, 128], F32, tag="qt")
        nc.tensor.transpose(mrow_ps[:, :], MF[:, g : g + 1], ident[:, :])
        mrow = small_p.tile([1, 128], F32, tag="mrowS")
        act.activation(mrow[:, :], mrow_ps[:, :], AF.Copy)
        mb_ps = ps_p.tile([128, 128], F32, tag="qt")
        nc.tensor.matmul(mb_ps[:, :], ones1[:, :], mrow[:, :], start=True,
                         stop=True)
        MB = small_p.tile([128, 128], F32, tag="MB")
        act.activation(MB[:, :], mb_ps[:, :], AF.Copy)
        # one-hot gather of the raw 9 pred values x 2 samples
        GAT = ps_p.tile([128, 9, 2], F32, tag="qt")
        MALL = mall_p.tile([128, NJ, 128], F32, tag="MALL")
        if int(os.environ.get("K_MBCAST", "0")):
            mb_v = MB[:, :].rearrange("p (o c) -> p o c", o=1)
            nj_v = NJCOL[:, :].rearrange("p (j o) -> p j o", o=1)
            mb_b, nj_b = bass.broadcast_tensor_aps(mb_v, nj_v)
            vec.tensor_tensor(MALL[:, :, :], mb_b, nj_b, OP.is_equal)
        else:
            for j in range(NJ):
                vec.tensor_scalar(MALL[:, j, :], MB[:, :],
                                  NJCOL[:, j : j + 1], None, OP.is_equal)
        for j in range(NJ):
            nc.tensor.matmul(
                GAT[:, :, :],
                MALL[:, j, :],
                XN[:, 2 * g : 2 * g + 2, j, :].rearrange("p s k -> p k s"),
                start=(j == 0),
                stop=(j == NJ - 1),
            )
        act.activation(GALL[:, g, :, :], GAT[:, :, :], AF.Copy)
        # duplicate-match count for conf pos dedup
        M3 = small_p.tile([128, 128], F32, tag="M3")
        vec.tensor_scalar(M3[:, :], MB[:, :], MF[:, g : g + 1], None,
                          OP.is_equal)
        vec.scalar_tensor_tensor(JK[:, :], M3[:, :], 0.0, SMASK[:, :],
                                 OP.add, OP.mult,
                                 accum_out=CB[:, g : g + 1])

    for g in range(NG):
        emit_score(g)
        if g >= 2:
            emit_gather(g - 2)
    emit_gather(NG - 2)
    emit_gather(NG - 1)

    # ---------------- losses ----------------
    OV = big_p.tile([128, 3], F32, tag="OV")
    vec.memset(OV[:, :], 0.0)

    # own-sample view of gathered values
    OWN = big_p.tile([128, NG, 9], F32, tag="OWN")
    vec.tensor_copy(OWN[0:64, :, :], GALL[0:64, :, :, 0])
    vec.tensor_copy(OWN[64:128, :, :], GALL[64:128, :, :, 1])

    # box loss: decode gathered boxes, smooth-l1 vs targets
    W2G = big_p.tile([128, NG], F32, tag="W2G")
    H2G = big_p.tile([128, NG], F32, tag="H2G")
    CXG = big_p.tile([128, NG], F32, tag="CXG")
    CYG = big_p.tile([128, NG], F32, tag="CYG")
    act.activation(W2G[:, :], OWN[:, :, 2], AF.Exp, bias=b_ln16)
    act.activation(H2G[:, :], OWN[:, :, 3], AF.Exp, bias=b_ln16)
    act.activation(CXG[:, :], OWN[:, :, 0], AF.Identity, bias=b_wneg,
                   scale=IMG_W)
    act.activation(CYG[:, :], OWN[:, :, 1], AF.Identity, bias=b_hneg,
                   scale=IMG_H)
    PC = big_p.tile([128, NG, 4], F32, tag="PC")
    vec.tensor_tensor(PC[:, :, 0], CXG[:, :], W2G[:, :], OP.subtract)
    vec.tensor_tensor(PC[:, :, 1], CYG[:, :], H2G[:, :], OP.subtract)
    vec.tensor_tensor(PC[:, :, 2], CXG[:, :], W2G[:, :], OP.add)
    vec.tensor_tensor(PC[:, :, 3], CYG[:, :], H2G[:, :], OP.add)
    D = big_p.tile([128, NG, 4], F32, tag="D")
    vec.tensor_tensor(D[:, :, :], PC[:, :, :], TGTC[:, :, :], OP.subtract)
    act.activation(D[:, :, :], D[:, :, :], AF.Abs)
    DM = big_p.tile([128, NG, 4], F32, tag="DM")
    vec.tensor_scalar(DM[:, :, :], D[:, :, :], 1.0, None, OP.min)
    Q1 = big_p.tile([128, NG, 4], F32, tag="Q1")
    Q2 = big_p.tile([128, NG, 4], F32, tag="Q2")
    act.activation(Q1[:, :, :], DM[:, :, :], AF.Square, scale=SQRT_HALF)
    act.activation(Q2[:, :, :], D[:, :, :], AF.Relu, bias=b_neg1)
    vec.scalar_tensor_tensor(
        JK[:, 0 : NG * 4].rearrange("p (g c) -> p g c", c=4),
        Q1[:, :, :], 0.0, Q2[:, :, :], OP.add, OP.add,
        accum_out=OV[:, 0:1],
    )

    # cls loss: logsumexp - logit[y] on gathered logits
    E = big_p.tile([128, NG, C], F32, tag="E")
    act.activation(E[:, :, :], OWN[:, :, 5:9], AF.Exp)
    SE = big_p.tile([128, NG], F32, tag="SE")
    vec.tensor_reduce(SE[:, :], E[:, :, :], AX.X, OP.add)
    LSE = big_p.tile([128, NG], F32, tag="LSE")
    act.activation(LSE[:, :], SE[:, :], AF.Ln)
    Y = big_p.tile([128, NG, C], F32, tag="Y")
    for cc in range(C):
        vec.tensor_scalar(Y[:, :, cc], TCF[:, :], float(cc), None, OP.is_equal)
    ZY = big_p.tile([128, NG, C], F32, tag="ZY")
    vec.tensor_tensor(ZY[:, :, :], OWN[:, :, 5:9], Y[:, :, :], OP.mult)
    SZY = big_p.tile([128, NG], F32, tag="SZY")
    vec.tensor_reduce(SZY[:, :], ZY[:, :, :], AX.X, OP.add)
    vec.scalar_tensor_tensor(
        JK[:, 0:NG], LSE[:, :], 0.0, SZY[:, :], OP.add, OP.subtract,
        accum_out=OV[:, 1:2],
    )

    # conf loss: sum softplus - sum conf over distinct matched preds
    KEEP = big_p.tile([128, NG], F32, tag="KEEP")
    vec.tensor_scalar(KEEP[:, :], CB[:, :], 0.0, None, OP.is_equal)
    XPC = big_p.tile([128, 1], F32, tag="XPC")
    vec.scalar_tensor_tensor(JK[:, 0:NG], OWN[:, :, 4], 0.0, KEEP[:, :],
                             OP.add, OP.mult, accum_out=XPC[:, :])
    vec.tensor_tensor(OV[:, 2:3], SPACC[:, :], XPC[:, :], OP.subtract)

    # cross-partition reduce -> [box, cls, conf]
    red_ps = ps_p.tile([3, 1], F32, tag="qt")
    nc.tensor.matmul(red_ps[:, :], OV[:, :], onescol[:, :], start=True,
                     stop=True)
    outs = small_p.tile([3, 1], F32, tag="outs")
    vec.tensor_copy(outs[:, :], red_ps[:, :])
    nc.sync.dma_start(out_d[:].rearrange("(x o) -> x o", o=1), outs[:, :])


_NC = None
LAST_RESULT = None


def _get_nc():
    global _NC
    if _NC is None:
        _NC = build_kernel()
    return _NC


def _selg():
    s = np.zeros((64, NG, 128), dtype=np.float32)
    for g in range(NG):
        s[2 * g, g, 0:64] = 1.0
        s[2 * g + 1, g, 64:128] = 1.0
    return s.reshape(64, NG * 128)


def kernel(predictions, target_boxes, target_classes):
    global LAST_RESULT
    nc = _get_nc()
    selg = _selg()
    in_maps = []
    for c in range(NCORES):
        sl = slice(c * BC, (c + 1) * BC)
        in_maps.append(
            {
                "predictions": np.ascontiguousarray(predictions[sl]),
                "target_boxes": np.ascontiguousarray(target_boxes[sl]),
                "target_classes": np.ascontiguousarray(target_classes[sl]),
                "selg": selg,
            }
        )
    LAST_RESULT = run_bass_kernel_spmd(nc, in_maps, list(range(NCORES)))
    res = LAST_RESULT.results
    box = np.float64(0.0)
    cls_ = np.float64(0.0)
    conf = np.float64(0.0)
    for c in range(NCORES):
        o = np.asarray(res[c]["out"], dtype=np.float64)
        box += o[0]
        cls_ += o[1]
        conf += o[2]
    total = (5.0 * box + 1.0 * cls_ + conf) / B
    return np.float32(total)
